# revision 1
# baseline (speedup 1.0000x reference)
"""Trainium2 Bass kernel for nn_Ocean_e2e: 48-step advection + 3x3 binomial smoothing.

Strategy: pure data-parallel over batch (B=8 samples -> 8 NeuronCores, one
1024x1024 grid per core). Whole state lives in SBUF in "block layout"
T_sb[p, b, w] = X[128*b + p, w]. Per step:

  Gx  = x-diff of T (unscaled; one-sided at w edges)        DVE, shifted APs
  P   = a * Gx   (a = -DT*ug*mask/dx-factors, host-folded)  DVE (in-place)
  Gy  = Dh @ T   (y-diff as banded matmul + corner corr)    PE (float32r)
  Q   = b * Gy   (b = -DT*vg*mask/dy-factors, host-folded)  DVE
  C   = Sh @ (T + P + Q)  (y-smooth, /16 folded; PSUM accum + corr)  PE
  u   = pairwise x-sums of C (1025-wide with raw end cols)  DVE
  Tn  = u + u_shift  (completes [1,2,1] x-smooth)           DVE

Y-direction one-sidedness and zero-pad boundaries live entirely in the Dh/Sh
block matrices, their corner-correction matrices, and the host-folded a/b
fields, so the hot loop has no boundary fixup work except two tiny DVE ops.

mask is all-ones by construction (spec fill=ones); it is still folded into
a and b, and applied to the final output on host for the tendency term.
"""

import numpy as np

DT = 600.0
STEPS = 48
R_EARTH = 6371000.0
DEG2RAD = np.pi / 180.0
B, H, W = 8, 1024, 1024
P = 128           # SBUF partitions
NB = H // P       # 8 row-blocks
N_CORES = 8

_cached = {}
LAST_EXEC_NS = None


# ----------------------------------------------------------------- host math
def _fields(ug, vg, lat, lon, mask):
    """Folded coefficient fields a, b (fp32) for one sample."""
    lat64 = lat.astype(np.float64)
    dlat = float(lat64[1] - lat64[0])
    dlon = float(lon.astype(np.float64)[1] - lon.astype(np.float64)[0])
    dy = R_EARTH * DEG2RAD * dlat
    dx = (R_EARTH * DEG2RAD * dlon) * np.cos(lat64 * DEG2RAD)  # [H]
    xfac = np.full((H, W), 0.5, np.float64) / dx[:, None]
    xfac[:, 0] = 1.0 / dx
    xfac[:, -1] = 1.0 / dx
    yfac = np.full((H, W), 0.5 / dy, np.float64)
    yfac[0, :] = 1.0 / dy
    yfac[-1, :] = 1.0 / dy
    m = mask.astype(np.float64)
    a = (-DT * ug.astype(np.float64) * m * xfac).astype(np.float32)
    b = (-DT * vg.astype(np.float64) * m * yfac).astype(np.float32)
    return a, b


def _block(x):
    """[H, W] -> SBUF block layout [P, NB, W]."""
    return np.ascontiguousarray(x.reshape(NB, P, W).transpose(1, 0, 2))


def _unblock(x):
    """[P, NB, W] -> [H, W]."""
    return np.ascontiguousarray(x.transpose(1, 0, 2).reshape(H, W))


def _matrices():
    """Per-block lhsT constants in SBUF layouts.

    dhT[p, b, m]  : (Dh_bb)^T          [128, 8, 128]
    shT[p, b, m]  : (Sh_bb)^T (/16)    [128, 8, 128]
    cdh[k, b, m]  : Dh corner corr     [16, 8, 128]  (G_T rows 0..13)
    csh[k, b, m]  : Sh corner corr     [48, 8, 128]  (G_T/G_P/G_Q at 0/16/32)
    G_T row i   (i=0..6):  X[128*(i+1)-1]   (last row of block i)
    G_T row 7+i (i=0..6):  X[128*(i+1)]     (first row of block i+1)
    """
    dh = np.zeros((NB, P, P), np.float32)   # dh[b] = Dh_bb (row-major operator)
    sh = np.zeros((NB, P, P), np.float32)
    for bb in range(NB):
        for p in range(P):
            h = bb * P + p
            # Dh: centered diff, one-sided at global edges; off-block via corr
            if h == 0:
                dh[bb, p, p] = -1.0
                dh[bb, p, p + 1] = 1.0
            elif h == H - 1:
                dh[bb, p, p - 1] = -1.0
                dh[bb, p, p] = 1.0
            else:
                if p - 1 >= 0:
                    dh[bb, p, p - 1] = -1.0
                if p + 1 < P:
                    dh[bb, p, p + 1] = 1.0
            # Sh: [1,2,1]/16, zero-pad at global edges; off-block via corr
            sh[bb, p, p] = 2.0 / 16.0
            if p - 1 >= 0:
                sh[bb, p, p - 1] = 1.0 / 16.0
            if p + 1 < P:
                sh[bb, p, p + 1] = 1.0 / 16.0
    dhT = np.ascontiguousarray(dh.transpose(2, 0, 1))  # [p=K, b, m]
    shT = np.ascontiguousarray(sh.transpose(2, 0, 1))

    cdh = np.zeros((16, NB, P), np.float32)
    csh = np.zeros((14, NB, P), np.float32)
    for bb in range(NB):
        if bb > 0:
            k = bb - 1            # G row for X[128*bb - 1]
            cdh[k, bb, 0] = -1.0
            csh[k, bb, 0] = 1.0 / 16.0
        if bb < NB - 1:
            k = 7 + bb            # G row for X[128*bb + 128]
            cdh[k, bb, P - 1] = 1.0
            csh[k, bb, P - 1] = 1.0 / 16.0
    # row-selector lhsT for G builds: G[i] = X[128*(i+1)-1] (i=0..6),
    # G[7+i] = X[128*(i+1)] (i=0..6); per block bb extracts rows 127 and 0
    lg = np.zeros((P, NB, 14), np.float32)
    for i in range(NB - 1):
        lg[127, i, i] = 1.0        # last row of block i -> G row i
        lg[0, i + 1, 7 + i] = 1.0  # first row of block i+1 -> G row 7+i
    # pack all constants into one [128, 8, 526] tensor (single DMA load):
    # dh 0:128 | sh 128:256 | cdh 256:384 (rows 0:16) | csh 384:512 (rows
    # 0:80) | lg 512:526
    consts = np.zeros((P, NB, 526), np.float32)
    consts[:, :, 0:128] = dhT
    consts[:, :, 128:256] = shT
    consts[0:16, :, 256:384] = cdh
    consts[0:14, :, 384:512] = csh
    consts[:, :, 512:526] = lg
    return consts


# ------------------------------------------------------------- bass program
def build_program(steps=STEPS):
    import concourse.mybir as mybir
    import concourse.tile as tile
    from concourse import bacc

    f32 = mybir.dt.float32
    f32r = mybir.dt.float32r

    nc = bacc.Bacc("TRN2", target_bir_lowering=False)
    # single packed input: t | a | b | consts along the free dim -> ONE load
    # DMA, so loop-body instructions wait on a single DMAHW sem lane.
    inp = nc.dram_tensor("inp", [P, NB, 3598], f32r, kind="ExternalInput")
    tout = nc.dram_tensor("tout", [P, NB, W], f32r, kind="ExternalOutput")

    NCHUNK = 2          # 512-wide matmul chunks per block
    CW = W // NCHUNK    # 512

    with tile.TileContext(nc) as tc:
        with (
            tc.tile_pool(name="state", bufs=1) as state,
            tc.tile_pool(name="psum_gy", bufs=2, space="PSUM") as pgy,
            tc.tile_pool(name="psum_c", bufs=2, space="PSUM") as pc,
            tc.tile_pool(name="psum_g", bufs=1, space="PSUM") as pg,
        ):
            m_sb = state.tile([P, NB, 3598], f32r)       # t | a | b | consts
            g_sb = state.tile([P, NB, W], f32r)          # Gx then P (in place)
            q_sb = state.tile([P, NB, W + 1], f32r)      # Q then u (1025 wide)
            gstT = state.tile([14, W], f32r)             # T corner rows
            gstPQ = state.tile([14, W], f32r)            # P+Q corner rows
            t_sb = m_sb[:, :, 0:W]
            a_sb = m_sb[:, :, W:2 * W]
            b_sb = m_sb[:, :, 2 * W:3 * W]
            co_sb = m_sb[:, :, 3 * W:3 * W + 526]

            nc.sync.dma_start(m_sb[:], inp[:])

            def body(_i):
                # --- gather T corner rows via DMA (idle DMA engines)
                nc.sync.dma_start(gstT[0:7, :], t_sb[127:128, 0:7, :])
                nc.sync.dma_start(gstT[7:14, :], t_sb[0:1, 1:8, :])

                # --- Gy = Dh @ T (+ corner corr) into PSUM, ACT-copy to q_sb
                for bb in range(NB):
                    for c in range(NCHUNK):
                        cs = slice(c * CW, (c + 1) * CW)
                        gy = pgy.tile([P, CW], f32, tag="gy")
                        nc.tensor.matmul(
                            gy[:], co_sb[:, bb, 0:128], t_sb[:, bb, cs],
                            start=True, stop=False)
                        nc.tensor.matmul(
                            gy[:], co_sb[0:14, bb, 256:384], gstT[:, cs],
                            start=False, stop=True)
                        nc.scalar.copy(q_sb[:, bb, cs], gy[:])

                # --- Gx (shifted diff), then per block: P = a*Gx and
                # Q = b*Gy (both in place), with the G_P+G_Q corner-row
                # build matmuls interleaved per block so the correction
                # fence collapses to a short tail after the last block.
                gp_pq = pg.tile([14, W], f32, tag="gpsum")
                for bb in range(NB):
                    nc.vector.tensor_sub(
                        g_sb[:, bb, 1:W - 1], t_sb[:, bb, 2:W],
                        t_sb[:, bb, 0:W - 2])
                    nc.vector.tensor_sub(
                        g_sb[:, bb, 0:W:W - 1],
                        t_sb[:, bb, 1:W:W - 2],
                        t_sb[:, bb, 0:W - 1:W - 2])
                    nc.vector.tensor_mul(
                        g_sb[:, bb, :], g_sb[:, bb, :], a_sb[:, bb, :])
                    nc.vector.tensor_mul(
                        q_sb[:, bb, 0:W], b_sb[:, bb, :], q_sb[:, bb, 0:W])
                    for c in range(NCHUNK):
                        cs = slice(c * CW, (c + 1) * CW)
                        nc.tensor.matmul(
                            gp_pq[:, cs], co_sb[:, bb, 512:526], g_sb[:, bb, cs],
                            start=(bb == 0), stop=False)
                        nc.tensor.matmul(
                            gp_pq[:, cs], co_sb[:, bb, 512:526], q_sb[:, bb, cs],
                            start=False, stop=(bb == NB - 1))
                nc.scalar.copy(gstPQ[:], gp_pq[:])

                # --- C = Sh@(T + P + Q) per block (+ stacked corr), then
                # --- u = pairwise x-sums of C into q_sb (1025 wide)
                for bb in range(NB):
                    ct = pc.tile([P, W], f32, tag="c")
                    for c in range(NCHUNK):
                        cs = slice(c * CW, (c + 1) * CW)
                        nc.tensor.matmul(
                            ct[:, cs], co_sb[:, bb, 128:256], t_sb[:, bb, cs],
                            start=True, stop=False)
                        nc.tensor.matmul(
                            ct[:, cs], co_sb[:, bb, 128:256], g_sb[:, bb, cs],
                            start=False, stop=False)
                        nc.tensor.matmul(
                            ct[:, cs], co_sb[:, bb, 128:256], q_sb[:, bb, cs],
                            start=False, stop=False)
                        nc.tensor.matmul(
                            ct[:, cs], co_sb[0:14, bb, 384:512], gstT[:, cs],
                            start=False, stop=False)
                        nc.tensor.matmul(
                            ct[:, cs], co_sb[0:14, bb, 384:512], gstPQ[:, cs],
                            start=False, stop=True)
                    # u slots 1..1024 <- C[0..1023] (ACT copy, PSUM one src)
                    nc.scalar.copy(q_sb[:, bb, 1:W + 1], ct[:])
                    # u slot 0 <- C[0]
                    nc.scalar.copy(q_sb[:, bb, 0:1], ct[:, 0:1])
                    # u slots 1..1023 += C[1..1023] -> u[w] = C[w-1]+C[w]
                    nc.vector.tensor_add(
                        q_sb[:, bb, 1:W], q_sb[:, bb, 1:W], ct[:, 1:W])

                # --- Tn = u + u_shift  (x-smooth complete; /16 already in Sh)
                for bb in range(NB):
                    nc.vector.tensor_add(
                        t_sb[:, bb, :], q_sb[:, bb, 0:W], q_sb[:, bb, 1:W + 1])

            with tc.For_i(0, steps,
                          hint_engines=(mybir.EngineType.PE,)) as _i:
                body(_i)

            nc.sync.dma_start(tout[:], t_sb[:])

    nc.finalize()
    return nc


# ------------------------------------------------------------------- driver
def kernel(T, ug, vg, lat, lon, mask):
    from concourse import bass_utils

    key = STEPS
    if key not in _cached:
        _cached[key] = build_program(STEPS)
    nc = _cached[key]

    consts = _matrices()
    in_maps = []
    for s in range(B):
        a, bfld = _fields(ug[s], vg[s], lat, lon, mask)
        packed = np.empty((P, NB, 3598), np.float32)
        packed[:, :, 0:W] = _block(T[s].astype(np.float32))
        packed[:, :, W:2 * W] = _block(a)
        packed[:, :, 2 * W:3 * W] = _block(bfld)
        packed[:, :, 3 * W:] = consts
        in_maps.append({"inp": packed})

    res = bass_utils.run_bass_kernel_spmd(nc, in_maps, core_ids=list(range(N_CORES)))
    global LAST_EXEC_NS
    if res.exec_time_ns is not None:
        LAST_EXEC_NS = res.exec_time_ns
    out = np.stack([_unblock(r["tout"]) for r in res.results])
    # mask is ones by spec; exact no-op then, but apply for safety
    return (out * mask[None].astype(np.float32)).astype(T.dtype)

